# revision 9
# baseline (speedup 1.0000x reference)
"""EncoderBlock kernel for 8 Trainium2 NeuronCores (data-parallel over batch).

Contract: kernel(**inputs) takes the FULL inputs of reference.setup_inputs()
and returns the FULL [16, 1024, 768] float32 output.

Strategy: pure data parallelism — 16 batches / 8 cores = 2 batches per core,
weights replicated, zero collectives. Per core a fused Bass/Tile program runs
LN1 -> QKV -> attention -> proj -> residual -> LN2 -> fc1/gelu -> fc2 ->
(normalized residual) -> LN3. Matmuls use float32r (full PE rate at
moving-dim >= 256, ~1e-4 rounding). LN1 gamma/beta fold into the QKV
weights/bias on the host; LN2/LN3 gamma/beta are applied on device.

Attention layout: Q^T,K^T are produced feature-major ([64, n] per head,
head pairs stacked on partitions), scores^T = K_h^T.T @ Q_h^T lands
keys-major so softmax needs no transposes; exp goes through the scalar
engine; P @ V uses a packed V tile ([V_h | ones] in a 128-wide, parity-
offset layout) so the softmax denominator rides along on an aligned spare
partition, then a K=1 matmul broadcasts 1/denom for the normalize.
"""

import sys

sys.path.insert(0, "/opt/trn_rl_repo")

import numpy as np

import concourse.bass as bass
import concourse.tile as tile
from concourse import mybir
from concourse.masks import make_identity
from concourse.vector_clock import ScopedClock, VectorClock
from concourse.bass_utils import run_bass_kernel_spmd

F32 = mybir.dt.float32
F32R = mybir.dt.float32r
AF = mybir.ActivationFunctionType
ALU = mybir.AluOpType

B, N, D = 16, 1024, 768
H, DH, HID = 12, 64, 3072
NCORES = 8
BLOC = B // NCORES
EPS = 1e-5
TC_N = N // 128   # 8 token tiles / batch
KC_D = D // 128   # 6 feature chunks
MC_H = HID // 128  # 24 hidden chunks


# ---------------------------------------------------------------------------
# Workarounds: this walrus build rejects >1 sync-wait command per instruction.
# ---------------------------------------------------------------------------
def _patched_drain_and_barrier(self, tick_clock, wait_clock):
    gc = tick_clock.global_clock
    n = len(gc)
    for i in range(n):
        t = gc[i]
        if t <= 0:
            continue
        vec = [0] * n
        vec[i] = t
        nop = self.nc.sync.nop(nofuse=True)
        wait_clock.add_sem_waits(nop.ins, ScopedClock({None: VectorClock(vec)}))
    self.nc.sync.drain()
    self.nc.all_engine_barrier()
    assert self.sems is not None
    popped = self.nc._tile_sem_poison_stack.pop()
    assert popped is self._sem_poison
    self.nc.clear_and_free_semaphores(list(self.sems.allocated().values()))
    self.nc.all_engine_barrier()


tile.TileContext._drain_and_barrier = _patched_drain_and_barrier


def _split_sync_waits(nc, limit=1):
    """Move excess per-instruction sync waits onto same-engine NoOps."""
    n_split = 0
    for fn in nc.m.functions:
        for bb in fn.blocks:
            out = []
            for ins in bb.instructions:
                si = ins.sync_info
                waits = list(si.on_wait) if (si and si.on_wait) else []
                if len(waits) > limit:
                    excess, keep = waits[:-limit], waits[-limit:]
                    for w in excess:
                        nop = mybir.InstNoOp(
                            name=f"{ins.name}-ws{n_split}",
                            engine=ins.engine,
                            ins=[],
                            outs=[],
                            sync_info=mybir.SyncInfo(on_wait=[w], on_update=[]),
                        )
                        n_split += 1
                        out.append(nop)
                    si.on_wait = keep
                out.append(ins)
            bb.instructions = out
    return n_split


# ---------------------------------------------------------------------------
# Device program (one core's share: BLOC batches)
# ---------------------------------------------------------------------------
def _layer_norm(nc, misc, x_t, eps_t):
    """(mean, rstd) [128,1] views for x_t [128, 768] f32."""
    xr = x_t.rearrange("p (s d) -> p s d", d=256)
    lt = misc.tile([128, 24], F32, tag="ln")
    stats = lt[:, 0:18].rearrange("p (s d) -> p s d", d=6)
    for s in range(3):
        nc.vector.bn_stats(out=stats[:, s, :], in_=xr[:, s, :])
    mv = lt[:, 18:20]
    nc.vector.bn_aggr(out=mv, in_=lt[:, 0:18])
    std = lt[:, 20:21]
    nc.scalar.activation(out=std, in_=mv[:, 1:2], func=AF.Sqrt, bias=eps_t)
    rstd = lt[:, 21:22]
    nc.vector.reciprocal(out=rstd, in_=std)
    return mv[:, 0:1], rstd


def _build_nc():
    nc = bass.Bass()

    x_d = nc.dram_tensor("x", [BLOC, N, D], F32, kind="ExternalInput")
    wqk_d = nc.dram_tensor("w_qk", [D, 2 * D], F32, kind="ExternalInput")
    wv_d = nc.dram_tensor("w_v", [D, D], F32, kind="ExternalInput")
    bqk_d = nc.dram_tensor("b_qk", [2 * D], F32, kind="ExternalInput")
    bv_d = nc.dram_tensor("b_v", [D], F32, kind="ExternalInput")
    pw_d = nc.dram_tensor("p_w", [D, D], F32, kind="ExternalInput")
    pb_d = nc.dram_tensor("p_b", [D], F32, kind="ExternalInput")
    w1_d = nc.dram_tensor("w1", [D, HID], F32, kind="ExternalInput")
    b1_d = nc.dram_tensor("b1", [HID], F32, kind="ExternalInput")
    w2_d = nc.dram_tensor("w2", [HID, D], F32, kind="ExternalInput")
    b2f_d = nc.dram_tensor("b2f", [D], F32, kind="ExternalInput")
    g2_d = nc.dram_tensor("g2", [D], F32, kind="ExternalInput")
    bt2_d = nc.dram_tensor("bt2", [D], F32, kind="ExternalInput")
    g3_d = nc.dram_tensor("g3", [D], F32, kind="ExternalInput")
    bt3_d = nc.dram_tensor("bt3", [D], F32, kind="ExternalInput")
    y_d = nc.dram_tensor("y", [BLOC, N, D], F32, kind="ExternalOutput")
    x2s_d = nc.dram_tensor("x2s", [BLOC, N, D], F32)

    with tile.TileContext(nc, pool_alloc_mode="queue") as tc:
        misc = tc.alloc_tile_pool(name="misc", bufs=2)
        const = tc.alloc_tile_pool(name="const", bufs=1)

        ident = const.tile([128, 128], F32)
        make_identity(nc, ident)
        eps_t = const.tile([128, 1], F32)
        nc.vector.memset(eps_t, EPS)
        ones_row = const.tile([1, 128], F32R)
        nc.vector.memset(ones_row.bitcast(F32), 1.0)
        onec = const.tile([128, TC_N, 6, 1], F32)
        nc.vector.memset(onec, 1.0)
        bqk_t = const.tile([128, 12], F32)
        nc.sync.dma_start(out=bqk_t, in_=bqk_d.rearrange("(c p) -> p c", p=128))
        b1_t = const.tile([128, MC_H], F32)
        nc.sync.dma_start(out=b1_t, in_=b1_d.rearrange("(c p) -> p c", p=128))
        bv_t = const.tile([128, D], F32)
        nc.sync.dma_start(out=bv_t, in_=bv_d[None, :].partition_broadcast(128))

        def load_bc(pool, dd, nm):
            t = pool.tile([128, D], F32, tag=nm, name=nm)
            nc.sync.dma_start(out=t, in_=dd[None, :].partition_broadcast(128))
            return t

        for b in range(BLOC):
            # Pool lifetimes are strictly nested (stack discipline):
            # pctx [start..E1] > pqkv [start..D] > pxnt [start..C] > phase pools
            pctx = tc.alloc_tile_pool(name=f"pctx{b}", bufs=1)
            pqkv = tc.alloc_tile_pool(name=f"pqkv{b}", bufs=1)

            # ---- Phase A: LN1 + transpose -> xnT ----
            pxnt = tc.alloc_tile_pool(name=f"pxnt{b}", bufs=1)
            xnT = pxnt.tile([128, KC_D, N], F32R)
            psA = tc.alloc_tile_pool(name=f"psA{b}", bufs=2, space="PSUM")
            for tcn in range(TC_N):
                x_t = misc.tile([128, D], F32, tag="x_in")
                nc.sync.dma_start(out=x_t, in_=x_d[b, tcn * 128:(tcn + 1) * 128, :])
                mean, rstd = _layer_norm(nc, misc, x_t, eps_t)
                xn = misc.tile([128, D], F32, tag="xout")
                nc.vector.tensor_scalar(out=xn, in0=x_t, scalar1=mean, scalar2=rstd,
                                        op0=ALU.subtract, op1=ALU.mult)
                for kc in range(KC_D):
                    pt = psA.tile([128, 128], F32, tag="tp")
                    nc.tensor.transpose(pt, xn[:, kc * 128:(kc + 1) * 128], ident)
                    nc.vector.tensor_copy(
                        out=xnT[:, kc, tcn * 128:(tcn + 1) * 128], in_=pt)
            psA.release()

            # ---- Phase B: Q,K projection (feature-major) ----
            qkT = pqkv.tile([128, 12, N], F32R, tag="qkT")
            pwqk = tc.alloc_tile_pool(name=f"pwqk{b}", bufs=1)
            psB = tc.alloc_tile_pool(name=f"psB{b}", bufs=3, space="PSUM")
            for half in range(2):  # Q columns then K columns of w_qk
                wqk = pwqk.tile([128, KC_D, D], F32R, tag="wqk")
                nc.gpsimd.dma_start(
                    out=wqk,
                    in_=wqk_d[:, half * D:(half + 1) * D].rearrange(
                        "(c p) n -> p c n", p=128))
                for fc6 in range(6):
                    fc = half * 6 + fc6
                    for nh in range(2):
                        ps = psB.tile([128, 512], F32, tag="qk")
                        for kc in range(KC_D):
                            nc.tensor.matmul(
                                ps,
                                wqk[:, kc, fc6 * 128:(fc6 + 1) * 128],
                                xnT[:, kc, nh * 512:(nh + 1) * 512],
                                start=(kc == 0), stop=(kc == KC_D - 1))
                        nc.vector.tensor_scalar(
                            out=qkT[:, fc, nh * 512:(nh + 1) * 512], in0=ps,
                            scalar1=bqk_t[:, fc:fc + 1], scalar2=None, op0=ALU.add)
            psB.release()
            pwqk.release()

            # ---- Phase C: V projection (token-major, packed [V_h | ones]) ----
            # per (token-chunk, head): even head -> V cols 0:64, ones col 64;
            # odd head -> ones col 0, V cols 64:128.  Ctx then lands on
            # partitions (h%2)*64..+64 and the denom on an aligned partition.
            V1 = pqkv.tile([128, TC_N, H, 128], F32R, tag="V1")
            nc.vector.memset(V1.bitcast(F32), 0.0)
            nc.vector.tensor_copy(out=V1[:, :, 0::2, 64:65], in_=onec)
            nc.vector.tensor_copy(out=V1[:, :, 1::2, 0:1], in_=onec)
            pwv = tc.alloc_tile_pool(name=f"pwv{b}", bufs=1)
            psC = tc.alloc_tile_pool(name=f"psC{b}", bufs=2, space="PSUM")
            wv = pwv.tile([128, KC_D, D], F32R)
            nc.gpsimd.dma_start(out=wv, in_=wv_d.rearrange("(c p) n -> p c n", p=128))
            for tcn in range(TC_N):
                ps = psC.tile([128, D], F32, tag="v")
                for kc in range(KC_D):
                    lhsT = xnT[:, kc, tcn * 128:(tcn + 1) * 128]
                    nc.tensor.matmul(ps[:, 0:512], lhsT, wv[:, kc, 0:512],
                                     start=(kc == 0), stop=(kc == KC_D - 1))
                    nc.tensor.matmul(ps[:, 512:768], lhsT, wv[:, kc, 512:768],
                                     start=(kc == 0), stop=(kc == KC_D - 1))
                pv3 = ps.rearrange("p (h d) -> p h d", d=DH)
                bv3 = bv_t.rearrange("p (h d) -> p h d", d=DH)
                nc.vector.tensor_tensor(
                    out=V1[:, tcn, 0::2, 0:64], in0=pv3[:, 0::2, :],
                    in1=bv3[:, 0::2, :], op=ALU.add)
                nc.vector.tensor_tensor(
                    out=V1[:, tcn, 1::2, 64:128], in0=pv3[:, 1::2, :],
                    in1=bv3[:, 1::2, :], op=ALU.add)
            psC.release()
            pwv.release()
            pxnt.release()

            # ---- Phase D: attention ----
            ctxT = pctx.tile([128, KC_D, N], F32R)
            pPT = tc.alloc_tile_pool(name=f"pPT{b}", bufs=2)
            psS = tc.alloc_tile_pool(name=f"psS{b}", bufs=3, space="PSUM")
            psX = tc.alloc_tile_pool(name=f"psX{b}", bufs=2, space="PSUM")
            for h in range(H):
                qh = (h % 2) * 64
                dh = 64 if h % 2 == 0 else 0  # denominator partition
                fq, fk = h // 2, 6 + h // 2
                for nh in range(2):
                    PT = pPT.tile([128, TC_N, 512], F32R, tag="PT")
                    for mc in range(TC_N):
                        sp = psS.tile([128, 512], F32, tag="sc")
                        nc.tensor.matmul(
                            sp,
                            qkT[qh:qh + 64, fk, mc * 128:(mc + 1) * 128],
                            qkT[qh:qh + 64, fq, nh * 512:(nh + 1) * 512],
                            start=True, stop=True)
                        nc.scalar.activation(out=PT[:, mc, :], in_=sp,
                                             func=AF.Exp, scale=0.125)
                    cp = psX.tile([128, 512], F32, tag="cp")
                    for mc in range(TC_N):
                        nc.tensor.matmul(cp, V1[:, mc, h, :], PT[:, mc, :],
                                         start=(mc == 0), stop=(mc == TC_N - 1))
                    rd = misc.tile([1, 512], F32R, tag="rd")
                    with nc.allow_low_precision(reason="f32r is fp32-width"):
                        nc.vector.reciprocal(out=rd, in_=cp[dh:dh + 1, :])
                    bb = psX.tile([128, 512], F32, tag="bb")
                    nc.tensor.matmul(bb, ones_row, rd, start=True, stop=True)
                    bsb = misc.tile([128, 512], F32, tag="bsb")
                    nc.scalar.copy(out=bsb[qh:qh + 64, :], in_=bb[qh:qh + 64, :])
                    nc.vector.tensor_tensor(
                        out=ctxT[qh:qh + 64, fq, nh * 512:(nh + 1) * 512],
                        in0=cp[qh:qh + 64, :], in1=bsb[qh:qh + 64, :],
                        op=ALU.mult)
            psX.release()
            psS.release()
            pPT.release()
            pqkv.release()

            # ---- Phase E1: proj + residual -> DRAM bounce ----
            ppw = tc.alloc_tile_pool(name=f"ppw{b}", bufs=1)
            pbe1 = tc.alloc_tile_pool(name=f"pbe1{b}", bufs=1)
            psE = tc.alloc_tile_pool(name=f"psE{b}", bufs=2, space="PSUM")
            pb_t = load_bc(pbe1, pb_d, f"pb{b}")
            pw = ppw.tile([128, KC_D, D], F32R)
            nc.gpsimd.dma_start(out=pw, in_=pw_d.rearrange("(c p) n -> p c n", p=128))
            for tcn in range(TC_N):
                ps = psE.tile([128, D], F32, tag="pj")
                for dc in range(KC_D):
                    lhsT = ctxT[:, dc, tcn * 128:(tcn + 1) * 128]
                    nc.tensor.matmul(ps[:, 0:512], lhsT, pw[:, dc, 0:512],
                                     start=(dc == 0), stop=(dc == KC_D - 1))
                    nc.tensor.matmul(ps[:, 512:768], lhsT, pw[:, dc, 512:768],
                                     start=(dc == 0), stop=(dc == KC_D - 1))
                x_t = misc.tile([128, D], F32, tag="x_in")
                nc.sync.dma_start(out=x_t, in_=x_d[b, tcn * 128:(tcn + 1) * 128, :])
                x2 = misc.tile([128, D], F32, tag="xwork")
                nc.vector.tensor_tensor(out=x2, in0=ps, in1=pb_t, op=ALU.add)
                nc.vector.tensor_tensor(out=x2, in0=x2, in1=x_t, op=ALU.add)
                nc.sync.dma_start(out=x2s_d[b, tcn * 128:(tcn + 1) * 128, :], in_=x2)
            psE.release()
            pbe1.release()
            ppw.release()
            pctx.release()

            # ---- Phase E2: LN2 + gamma/beta + transpose ----
            pff = tc.alloc_tile_pool(name=f"pff{b}", bufs=1)
            pffh = tc.alloc_tile_pool(name=f"pffh{b}", bufs=1)
            pft = tc.alloc_tile_pool(name=f"pft{b}", bufs=1)
            ff_in = pff.tile([128, TC_N, D], F32)
            ffinT = pft.tile([128, KC_D, N], F32R)
            pbe2 = tc.alloc_tile_pool(name=f"pbe2{b}", bufs=1)
            psT = tc.alloc_tile_pool(name=f"psT{b}", bufs=2, space="PSUM")
            g2_t = load_bc(pbe2, g2_d, f"g2{b}")
            bt2_t = load_bc(pbe2, bt2_d, f"bt2{b}")
            for tcn in range(TC_N):
                x2 = misc.tile([128, D], F32, tag="x_in")
                nc.sync.dma_start(out=x2, in_=x2s_d[b, tcn * 128:(tcn + 1) * 128, :])
                mean, rstd = _layer_norm(nc, misc, x2, eps_t)
                fi = ff_in[:, tcn, :]
                nc.vector.tensor_scalar(out=fi, in0=x2, scalar1=mean, scalar2=rstd,
                                        op0=ALU.subtract, op1=ALU.mult)
                nc.vector.tensor_tensor(out=fi, in0=fi, in1=g2_t, op=ALU.mult)
                nc.vector.tensor_tensor(out=fi, in0=fi, in1=bt2_t, op=ALU.add)
                for kc in range(KC_D):
                    pt = psT.tile([128, 128], F32, tag="tp2")
                    nc.tensor.transpose(pt, fi[:, kc * 128:(kc + 1) * 128], ident)
                    nc.vector.tensor_copy(
                        out=ffinT[:, kc, tcn * 128:(tcn + 1) * 128], in_=pt)
            psT.release()
            pbe2.release()

            # ---- Phase F: fc1 + gelu (feature-major out) ----
            ffhT = pffh.tile([128, MC_H, N], F32R)
            pw1 = tc.alloc_tile_pool(name=f"pw1{b}", bufs=2)
            psF = tc.alloc_tile_pool(name=f"psF{b}", bufs=3, space="PSUM")
            for mcg in range(MC_H // 2):
                w1b = pw1.tile([128, KC_D, 256], F32R, tag="w1b")
                nc.gpsimd.dma_start(
                    out=w1b,
                    in_=w1_d[:, mcg * 256:(mcg + 1) * 256].rearrange(
                        "(c p) n -> p c n", p=128))
                for mi in range(2):
                    mc = mcg * 2 + mi
                    for nh in range(2):
                        ps = psF.tile([128, 512], F32, tag="f1")
                        for kc in range(KC_D):
                            nc.tensor.matmul(
                                ps,
                                w1b[:, kc, mi * 128:(mi + 1) * 128],
                                ffinT[:, kc, nh * 512:(nh + 1) * 512],
                                start=(kc == 0), stop=(kc == KC_D - 1))
                        nc.scalar.activation(
                            out=ffhT[:, mc, nh * 512:(nh + 1) * 512], in_=ps,
                            func=AF.Gelu, bias=b1_t[:, mc:mc + 1])
            psF.release()
            pw1.release()

            # ---- Phase G: fc2 (streamed weights, 4-token-group PSUM) ----
            pw2 = tc.alloc_tile_pool(name=f"pw2{b}", bufs=3)
            pbe3 = tc.alloc_tile_pool(name=f"pbe3{b}", bufs=1)
            psG = tc.alloc_tile_pool(name=f"psG{b}", bufs=4, space="PSUM")
            b2f_t = load_bc(pbe3, b2f_d, f"b2f{b}")
            g3_t = load_bc(pbe3, g3_d, f"g3{b}")
            bt3_t = load_bc(pbe3, bt3_d, f"bt3{b}")
            for tg in range(2):
                pss = [psG.tile([128, D], F32, tag="f2", name=f"f2_{tg}_{i}")
                       for i in range(4)]
                for kc in range(MC_H):
                    w2t = pw2.tile([128, D], F32R, tag="w2t")
                    nc.gpsimd.dma_start(
                        out=w2t, in_=w2_d[kc * 128:(kc + 1) * 128, :])
                    for ti in range(4):
                        tcn = tg * 4 + ti
                        lhsT = ffhT[:, kc, tcn * 128:(tcn + 1) * 128]
                        nc.tensor.matmul(pss[ti][:, 0:512], lhsT, w2t[:, 0:512],
                                         start=(kc == 0), stop=(kc == MC_H - 1))
                        nc.tensor.matmul(pss[ti][:, 512:768], lhsT, w2t[:, 512:768],
                                         start=(kc == 0), stop=(kc == MC_H - 1))
                for ti in range(4):
                    tcn = tg * 4 + ti
                    x3 = misc.tile([128, D], F32, tag="xwork")
                    nc.vector.tensor_tensor(out=x3, in0=pss[ti], in1=b2f_t,
                                            op=ALU.add)
                    nc.vector.tensor_tensor(out=x3, in0=x3, in1=ff_in[:, tcn, :],
                                            op=ALU.add)
                    mean, rstd = _layer_norm(nc, misc, x3, eps_t)
                    yt = misc.tile([128, D], F32, tag="xout")
                    nc.vector.tensor_scalar(out=yt, in0=x3, scalar1=mean,
                                            scalar2=rstd,
                                            op0=ALU.subtract, op1=ALU.mult)
                    nc.vector.tensor_tensor(out=yt, in0=yt, in1=g3_t,
                                            op=ALU.mult)
                    nc.vector.tensor_tensor(out=yt, in0=yt, in1=bt3_t,
                                            op=ALU.add)
                    nc.sync.dma_start(out=y_d[b, tcn * 128:(tcn + 1) * 128, :],
                                      in_=yt)
            psG.release()
            pbe3.release()
            pw2.release()
            pft.release()
            pffh.release()
            pff.release()

        const.release()
        misc.release()

    _split_sync_waits(nc)
    return nc


_NC_CACHE = {}


def _get_nc():
    if "nc" not in _NC_CACHE:
        _NC_CACHE["nc"] = _build_nc()
    return _NC_CACHE["nc"]


def kernel(x, ln1_g, ln1_b, qkv_w, qkv_b, proj_w, proj_b,
           ln2_g, ln2_b, fc1_w, fc1_b, fc2_w, fc2_b, ln3_g, ln3_b,
           **extra):
    x = np.asarray(x, np.float32)
    f = lambda a: np.ascontiguousarray(np.asarray(a, np.float32))
    qkv_w, qkv_b = f(qkv_w), f(qkv_b)

    # Fold LN1 gamma/beta into QKV weights/bias (host, fp32).
    w_eff = np.asarray(ln1_g, np.float32)[:, None] * qkv_w
    b_eff = np.asarray(ln1_b, np.float32) @ qkv_w + qkv_b

    common = {
        "w_qk": f(w_eff[:, :2 * D]),
        "w_v": f(w_eff[:, 2 * D:]),
        "b_qk": f(b_eff[:2 * D]),
        "b_v": f(b_eff[2 * D:]),
        "p_w": f(proj_w), "p_b": f(proj_b),
        "w1": f(fc1_w), "b1": f(fc1_b),
        "w2": f(fc2_w), "b2f": f(fc2_b),
        "g2": f(ln2_g), "bt2": f(ln2_b),
        "g3": f(ln3_g), "bt3": f(ln3_b),
    }
    in_maps = [dict(common, x=f(x[i * BLOC:(i + 1) * BLOC])) for i in range(NCORES)]

    nc = _get_nc()
    res = run_bass_kernel_spmd(nc, in_maps, core_ids=list(range(NCORES)))
    _NC_CACHE["last_result"] = res
    return np.concatenate([r["y"] for r in res.results], axis=0)
